# revision 28
# baseline (speedup 1.0000x reference)
"""Multi-head attention (B=2, S=2048, D=1024, H=16) on 8 TRN2 NeuronCores.

Sharding: core c handles batch b = c//4 and head-group g = c%4 (4 heads,
d-slice of 256). All on-chip data is fp16 (PE rate is identical to f32r at
N>=256, but DMA and SBUF cost halve); matmul accumulation stays fp32 in PSUM.

Per core:
  KT = (WkT.T @ Xkv^T + bk)        [256, SKV]  fp16
  V  = Xkv^T.T-chunks @ WvT        [SKV, 256]  fp16 ([k, d] layout + valid col)
  QT = (WqT.T @ X^T + bq)          [256, 2048] fp16
  per (head h, q-block of 512):
    per kc (128 keys): pss = K_h Q_h^T (PSUM), P16 = exp(pss/8) (Act, fp16)
    psO [65, 512] accumulates [V_h | valid].T P16 over kc -> unnormalized O^T
      (row 64 = softmax denominator over valid keys)
    OT = psO[0:64] * recip(den)  (DVE recip + GpSimd partition broadcast)
  OUT[qc, :] = OT.T @ WoT  per 128-row q-chunk, fp16 out, DMA per 512-col half

The emission order software-pipelines the engines: Q-projection of the next
q-block and the output projection of the previous q-block are interleaved
between attention matmuls so the PE never idles while the Act engine works
through the exp stream.

Host side: keys/values are compacted by the attention mask (exact: masked
keys contribute exp->0 in the fp32 reference), padded to a multiple of 128;
the valid-flag column excludes padding from numerator and denominator.
V/O biases fold into a host-side constant: A@(V+bv)Wo^T + bo = A@V@Wo^T +
(bv@Wo^T + bo). Partial outputs over head-groups are summed on the host.
"""

import math
import os
from functools import lru_cache

import numpy as np

D_MODEL = 1024
NUM_HEADS = 16
D_K = 64
B = 2
S = 2048
N_CORES = 8
GROUPS = 4          # head-groups = cores per batch
DH = 256            # d-slice per core (4 heads x 64)
NH_LOC = 4          # heads per core
P = 128
CC = D_MODEL // P   # contraction chunks
QB = S // 512       # q blocks

# results of the last hardware run (BassKernelResults), for test harnesses
last_results = None


@lru_cache(maxsize=2)
def _build(SKV: int):
    import concourse.mybir as mybir
    import concourse.tile as tile
    from concourse import bacc

    f32 = mybir.dt.float32
    f16 = mybir.dt.float16
    KC = SKV // P
    kbs = [(s0, min(512, SKV - s0)) for s0 in range(0, SKV, 512)]

    nc = bacc.Bacc("TRN2", target_bir_lowering=False, debug=False,
                   num_devices=N_CORES)

    # All inputs are pre-arranged on the host to the exact SBUF layout, so
    # every DMA is 128 fat contiguous rows (descriptor generation on the
    # Sync engine is proportional to row count).
    XT_d = nc.dram_tensor("xt", [QB, P, CC, 512], f16, kind="ExternalInput").ap()
    XKV_d = nc.dram_tensor("xkv", [P, CC, SKV], f16, kind="ExternalInput").ap()
    WQT_d = nc.dram_tensor("wqt", [2, P, CC, P], f16, kind="ExternalInput").ap()
    WKT_d = nc.dram_tensor("wkt", [P, CC, DH], f16, kind="ExternalInput").ap()
    WVT_d = nc.dram_tensor("wvt", [P, CC, DH], f16, kind="ExternalInput").ap()
    WOT_d = nc.dram_tensor("wot", [P, 2, D_MODEL], f16, kind="ExternalInput").ap()
    bq_d = nc.dram_tensor("bq", [P, 2], f32, kind="ExternalInput").ap()
    bk_d = nc.dram_tensor("bk", [P, 2], f32, kind="ExternalInput").ap()
    vf_d = nc.dram_tensor("vf", [P, KC], f32, kind="ExternalInput").ap()
    OUT_d = nc.dram_tensor("out", [S, D_MODEL], f16, kind="ExternalOutput").ap()
    debug = bool(os.environ.get("KERNEL_DEBUG"))
    if debug:
        dKT = nc.dram_tensor("dbg_kt", [P, 2, SKV], f16, kind="ExternalOutput").ap()
        dQT = nc.dram_tensor("dbg_qt", [P, 2, S], f16, kind="ExternalOutput").ap()
        dV = nc.dram_tensor("dbg_v", [P, KC, NH_LOC, 65], f16, kind="ExternalOutput").ap()
        dOT = nc.dram_tensor("dbg_ot", [P, 2, S], f16, kind="ExternalOutput").ap()
        dWK = nc.dram_tensor("dbg_wk", [P, CC, DH], f16, kind="ExternalOutput").ap()
        dWV = nc.dram_tensor("dbg_wv", [P, CC, DH], f16, kind="ExternalOutput").ap()
        dWO = nc.dram_tensor("dbg_wo", [P, 2, D_MODEL], f16, kind="ExternalOutput").ap()
        dXKV = nc.dram_tensor("dbg_xkv", [P, CC, SKV], f16, kind="ExternalOutput").ap()

    with tile.TileContext(nc) as tc:
        with tc.tile_pool(name="res", bufs=1) as res:
            XT_sb = res.tile([P, QB, CC, 512], f16)
            XKV_sb = res.tile([P, CC, SKV], f16)
            WQT_sb = res.tile([P, 2, CC, P], f16)
            WKT_sb = res.tile([P, CC, DH], f16)
            WVT_sb = res.tile([P, CC, DH], f16)
            WOT_sb = res.tile([P, 2, D_MODEL], f16)
            bq_sb = res.tile([P, 2], f32)
            bk_sb = res.tile([P, 2], f32)
            QT_sb = res.tile([P, 2, S], f16)
            KT_sb = res.tile([P, 2, SKV], f16)
            V_sb = res.tile([P, KC, NH_LOC, 65], f16)
            P_sb = res.tile([P, 2, KC, 512], f16)   # parity-double-buffered
            OT_sb = res.tile([P, 2, S], f16)
            vf_sb = res.tile([P, KC], f32)
            ones4 = res.tile([P, NH_LOC, 1], f16)
            nc.vector.memset(ones4[:], 1.0)

            # First wave of DMAs only: what Q-projection (t=0) of block 0
            # needs. Consumers appear to wait on all DMA traffic issued
            # before them, so later inputs are issued just-in-time below.
            nc.sync.dma_start(WQT_sb[:, 0], WQT_d[0])
            nc.sync.dma_start(bq_sb[:], bq_d)
            nc.sync.dma_start(XT_sb[:, 0], XT_d[0])
            nc.sync.dma_start(WQT_sb[:, 1], WQT_d[1])

            with tc.tile_pool(name="osb", bufs=4) as osb, \
                 tc.tile_pool(name="nrm", bufs=4) as nrm, \
                 tc.tile_pool(name="psS", bufs=2, space="PSUM") as psS, \
                 tc.tile_pool(name="psO", bufs=2, space="PSUM") as psO, \
                 tc.tile_pool(name="gen", bufs=2, space="PSUM") as gen:

                # ---------- filler unit emitters ----------
                def emit_qproj(qb, t):
                    psq = gen.tile([P, 512], f32, tag="g")
                    for cc in range(CC):
                        nc.tensor.matmul(
                            psq[:],
                            WQT_sb[:, t, cc, :],
                            XT_sb[:, qb, cc, :],
                            start=(cc == 0), stop=(cc == CC - 1))
                    nc.vector.tensor_scalar_add(
                        QT_sb[:, t, qb * 512:(qb + 1) * 512], psq[:],
                        bq_sb[:, t:t + 1])

                def emit_kproj(k0, sz, t):
                    psk = gen.tile([P, 512], f32, tag="g")
                    for cc in range(CC):
                        nc.tensor.matmul(
                            psk[:, :sz],
                            WKT_sb[:, cc, t * P:(t + 1) * P],
                            XKV_sb[:, cc, k0:k0 + sz],
                            start=(cc == 0), stop=(cc == CC - 1))
                    nc.vector.tensor_scalar_add(
                        KT_sb[:, t, k0:k0 + sz], psk[:, :sz], bk_sb[:, t:t + 1])

                def emit_vproj(kc):
                    psv = gen.tile([P, 512], f32, tag="g")
                    for cc in range(CC):
                        nc.tensor.matmul(
                            psv[:, :DH],
                            XKV_sb[:, cc, kc * P:(kc + 1) * P],
                            WVT_sb[:, cc, :],
                            start=(cc == 0), stop=(cc == CC - 1))
                    nc.vector.tensor_copy(
                        V_sb[:, kc, :, 0:64],
                        psv[:, :DH].rearrange("p (h d) -> p h d", h=NH_LOC))
                    # valid-flag column via DVE (a direct DMA into the
                    # interleaved stride-65 slots clobbers neighboring V
                    # elements: DMA write granule > element size)
                    nc.vector.tensor_scalar_mul(
                        V_sb[:, kc, :, 64:65], ones4[:], vf_sb[:, kc:kc + 1])

                def emit_ph3(qb, qi, nb):
                    # output chunk [128 q, 512 d] for q-chunk qc of block qb
                    qc = qb * 4 + qi
                    ps3 = gen.tile([P, 512], f32, tag="g")
                    for t in range(2):
                        nc.tensor.matmul(
                            ps3[:],
                            OT_sb[:, t, qc * P:(qc + 1) * P],
                            WOT_sb[:, t, nb * 512:(nb + 1) * 512],
                            start=(t == 0), stop=(t == 1))
                    ob = osb.tile([P, 512], f16, tag="ob")
                    nc.vector.tensor_copy(ob[:], ps3[:])
                    nc.sync.dma_start(
                        OUT_d[qc * P:(qc + 1) * P, nb * 512:(nb + 1) * 512],
                        ob[:])

                # ---------- attention unit ----------
                def emit_unit(h, qb, fillers):
                    """fillers: list of thunks to emit between scores and AV."""
                    t, po = h // 2, (h % 2) * 64
                    par = (qb * NH_LOC + h) % 2
                    q0 = qb * 512
                    # kc pairs so each Act exp instruction covers 2 kc tiles
                    prs = [list(range(j, min(j + 2, KC)))
                           for j in range(0, KC, 2)]

                    def emit_sc(pair):
                        pss = psS.tile([P, 2, 512], f32, tag="s")
                        for j, kc in enumerate(pair):
                            nc.tensor.matmul(
                                pss[:, j, :],
                                KT_sb[po:po + 64, t, kc * P:(kc + 1) * P],
                                QT_sb[po:po + 64, t, q0:q0 + 512],
                                start=True, stop=True)
                        nc.scalar.activation(
                            P_sb[:, par, pair[0]:pair[0] + len(pair), :],
                            pss[:, 0:len(pair), :],
                            mybir.ActivationFunctionType.Exp, scale=0.125)

                    for pair in prs[:2]:
                        emit_sc(pair)
                    for f in fillers:
                        f()
                    for pair in prs[2:]:
                        emit_sc(pair)

                    pso = psO.tile([65, 512], f32, tag="o")
                    for kc in range(KC):
                        nc.tensor.matmul(
                            pso[:],
                            V_sb[:, kc, h, :],
                            P_sb[:, par, kc, :],
                            start=(kc == 0), stop=(kc == KC - 1))
                    den = nrm.tile([1, 512], f32, tag="den")
                    nc.vector.tensor_copy(den[:], pso[64:65, :])
                    rec = nrm.tile([1, 512], f32, tag="rec")
                    nc.vector.reciprocal_approx_fast(rec[:], den[:])
                    recb = nrm.tile([64, 512], f32, tag="recb")
                    nc.gpsimd.partition_broadcast(recb[:], rec[:], channels=64)
                    nc.vector.tensor_mul(
                        OT_sb[po:po + 64, t, q0:q0 + 512], pso[0:64, :], recb[:])

                # ---------- lead (DMA issues interleaved just-in-time) ----
                nc.sync.dma_start(WKT_sb[:], WKT_d)
                nc.sync.dma_start(bk_sb[:], bk_d)
                nc.sync.dma_start(XKV_sb[:], XKV_d)
                for t in range(2):
                    emit_qproj(0, t)
                nc.sync.dma_start(WVT_sb[:], WVT_d)
                nc.sync.dma_start(vf_sb[:], vf_d)
                for t in range(2):
                    for (k0, sz) in kbs:
                        emit_kproj(k0, sz, t)
                nc.sync.dma_start(WOT_sb[:], WOT_d)
                nc.sync.dma_start(XT_sb[:, 1], XT_d[1])
                for kc in range(KC):
                    emit_vproj(kc)
                for qb in range(2, QB):
                    nc.sync.dma_start(XT_sb[:, qb], XT_d[qb])

                # ---------- main loop ----------
                for qb in range(QB):
                    # build filler thunks for this q-block
                    fillers = []
                    if qb + 1 < QB:
                        for t in range(2):
                            fillers.append(
                                lambda qb=qb, t=t: emit_qproj(qb + 1, t))
                    if qb >= 1:
                        for qi in range(4):
                            for nb in range(2):
                                fillers.append(
                                    lambda qb=qb, qi=qi, nb=nb:
                                        emit_ph3(qb - 1, qi, nb))
                    # spread fillers across the 4 head-units
                    nf = len(fillers)
                    for h in range(NH_LOC):
                        lo = nf * h // NH_LOC
                        hi = nf * (h + 1) // NH_LOC
                        emit_unit(h, qb, fillers[lo:hi])

                # tail: output projection of the last q-block
                for qi in range(4):
                    for nb in range(2):
                        emit_ph3(QB - 1, qi, nb)

                if debug:
                    nc.sync.dma_start(dKT, KT_sb[:])
                    nc.sync.dma_start(dQT, QT_sb[:])
                    nc.sync.dma_start(dV, V_sb[:])
                    nc.sync.dma_start(dOT, OT_sb[:])
                    nc.sync.dma_start(dWK, WKT_sb[:])
                    nc.sync.dma_start(dWV, WVT_sb[:])
                    nc.sync.dma_start(dWO, WOT_sb[:])
                    nc.sync.dma_start(dXKV, XKV_sb[:])

    nc.compile()
    return nc


def kernel(X, mask, W_Q, b_Q, W_K, b_K, W_V, b_V, W_O, b_O):
    global last_results
    from concourse.bass_utils import run_bass_kernel_spmd

    X = np.asarray(X, dtype=np.float32)
    mask2 = np.asarray(mask).reshape(B, S) != 0
    counts = mask2.sum(axis=1)
    assert counts.min() >= 1
    SKV = max(P, int(math.ceil(counts.max() / P)) * P)

    KC = SKV // P
    XT16 = np.ascontiguousarray(X.transpose(0, 2, 1)).astype(np.float16)
    XKV16 = np.zeros((B, D_MODEL, SKV), dtype=np.float16)
    VF32 = np.zeros((B, SKV), dtype=np.float32)
    for b in range(B):
        idx = np.nonzero(mask2[b])[0]
        XKV16[b, :, :len(idx)] = XT16[b][:, idx]
        VF32[b, :len(idx)] = 1.0

    nc = _build(SKV)

    # rearrange to the SBUF layouts (fat contiguous DMA rows)
    def pcd(w):     # [D, n] -> [P, CC, n]
        return np.ascontiguousarray(
            w.reshape(CC, P, w.shape[1]).transpose(1, 0, 2))

    xt_h = [np.ascontiguousarray(
        XT16[b].reshape(CC, P, QB, 512).transpose(2, 1, 0, 3)) for b in range(B)]
    xkv_h = [pcd(XKV16[b]) for b in range(B)]
    vf_h = [np.ascontiguousarray(VF32[b].reshape(KC, P).T) for b in range(B)]

    in_maps = []
    for c in range(N_CORES):
        b, g = divmod(c, GROUPS)
        sl = slice(g * DH, (g + 1) * DH)
        in_maps.append({
            "xt": xt_h[b],
            "xkv": xkv_h[b],
            "wqt": np.ascontiguousarray(
                W_Q[sl, :].T.astype(np.float16)
                .reshape(CC, P, 2, P).transpose(2, 1, 0, 3)),
            "wkt": pcd(W_K[sl, :].T.astype(np.float16)),
            "wvt": pcd(W_V[sl, :].T.astype(np.float16)),
            "wot": np.ascontiguousarray(
                W_O[:, sl].T.astype(np.float16).reshape(2, P, D_MODEL)
                .transpose(1, 0, 2)),
            "bq": np.ascontiguousarray(
                b_Q[sl].astype(np.float32).reshape(2, P).T),
            "bk": np.ascontiguousarray(
                b_K[sl].astype(np.float32).reshape(2, P).T),
            "vf": vf_h[b],
        })

    trace_cores = None
    if os.environ.get("BASS_TRACE"):
        trace_cores = [int(x) for x in
                       os.environ.get("BASS_TRACE_CORES", "0").split(",")]
    res = run_bass_kernel_spmd(nc, in_maps, core_ids=list(range(N_CORES)),
                               trace_cores=trace_cores)
    last_results = res

    const = np.asarray(b_V, np.float64) @ np.asarray(W_O, np.float64).T \
        + np.asarray(b_O, np.float64)
    out = np.zeros((B, S, D_MODEL), dtype=np.float64)
    for c in range(N_CORES):
        b = c // GROUPS
        out[b] += res.results[c]["out"].astype(np.float64)
    out += const[None, None, :]
    return out.astype(np.float32)


# revision 31
# speedup vs baseline: 1.0074x; 1.0074x over previous
"""Multi-head attention (B=2, S=2048, D=1024, H=16) on 8 TRN2 NeuronCores.

Sharding: core c handles batch b = c//4 and head-group g = c%4 (4 heads,
d-slice of 256). All on-chip data is fp16 (PE rate is identical to f32r at
N>=256, but DMA and SBUF cost halve); matmul accumulation stays fp32 in PSUM.

Per core:
  KT = (WkT.T @ Xkv^T + bk)        [256, SKV]  fp16
  V  = Xkv^T.T-chunks @ WvT        [SKV, 256]  fp16 ([k, d] layout + valid col)
  QT = (WqT.T @ X^T + bq)          [256, 2048] fp16
  per (head h, q-block of 512):
    per kc (128 keys): pss = K_h Q_h^T (PSUM), P16 = exp(pss/8) (Act, fp16)
    psO [65, 512] accumulates [V_h | valid].T P16 over kc -> unnormalized O^T
      (row 64 = softmax denominator over valid keys)
    OT = psO[0:64] * recip(den)  (DVE recip + GpSimd partition broadcast)
  OUT[qc, :] = OT.T @ WoT  per 128-row q-chunk, fp16 out, DMA per 512-col half

The emission order software-pipelines the engines: Q-projection of the next
q-block and the output projection of the previous q-block are interleaved
between attention matmuls so the PE never idles while the Act engine works
through the exp stream.

Host side: keys/values are compacted by the attention mask (exact: masked
keys contribute exp->0 in the fp32 reference), padded to a multiple of 128;
the valid-flag column excludes padding from numerator and denominator.
V/O biases fold into a host-side constant: A@(V+bv)Wo^T + bo = A@V@Wo^T +
(bv@Wo^T + bo). Partial outputs over head-groups are summed on the host.
"""

import math
import os
from functools import lru_cache

import numpy as np

D_MODEL = 1024
NUM_HEADS = 16
D_K = 64
B = 2
S = 2048
N_CORES = 8
GROUPS = 4          # head-groups = cores per batch
DH = 256            # d-slice per core (4 heads x 64)
NH_LOC = 4          # heads per core
P = 128
CC = D_MODEL // P   # contraction chunks
QB = S // 512       # q blocks

# results of the last hardware run (BassKernelResults), for test harnesses
last_results = None


@lru_cache(maxsize=2)
def _build(SKV: int):
    import concourse.mybir as mybir
    import concourse.tile as tile
    from concourse import bacc

    f32 = mybir.dt.float32
    f16 = mybir.dt.float16
    KC = SKV // P
    kbs = [(s0, min(512, SKV - s0)) for s0 in range(0, SKV, 512)]

    nc = bacc.Bacc("TRN2", target_bir_lowering=False, debug=False,
                   num_devices=N_CORES)

    # All inputs are pre-arranged on the host to the exact SBUF layout, so
    # every DMA is 128 fat contiguous rows (descriptor generation on the
    # Sync engine is proportional to row count).
    XT_d = nc.dram_tensor("xt", [QB, P, CC, 512], f16, kind="ExternalInput").ap()
    XKV_d = nc.dram_tensor("xkv", [P, CC, SKV], f16, kind="ExternalInput").ap()
    WQT_d = nc.dram_tensor("wqt", [2, P, CC, P], f16, kind="ExternalInput").ap()
    WKT_d = nc.dram_tensor("wkt", [P, CC, DH], f16, kind="ExternalInput").ap()
    WVT_d = nc.dram_tensor("wvt", [P, CC, DH], f16, kind="ExternalInput").ap()
    WOT_d = nc.dram_tensor("wot", [P, 2, D_MODEL], f16, kind="ExternalInput").ap()
    bq_d = nc.dram_tensor("bq", [P, 2], f32, kind="ExternalInput").ap()
    bk_d = nc.dram_tensor("bk", [P, 2], f32, kind="ExternalInput").ap()
    vf_d = nc.dram_tensor("vf", [P, KC], f32, kind="ExternalInput").ap()
    OUT_d = nc.dram_tensor("out", [S, D_MODEL], f16, kind="ExternalOutput").ap()
    debug = bool(os.environ.get("KERNEL_DEBUG"))
    if debug:
        dKT = nc.dram_tensor("dbg_kt", [P, 2, SKV], f16, kind="ExternalOutput").ap()
        dQT = nc.dram_tensor("dbg_qt", [P, 2, S], f16, kind="ExternalOutput").ap()
        dV = nc.dram_tensor("dbg_v", [P, KC, NH_LOC, 65], f16, kind="ExternalOutput").ap()
        dOT = nc.dram_tensor("dbg_ot", [P, 2, S], f16, kind="ExternalOutput").ap()
        dWK = nc.dram_tensor("dbg_wk", [P, CC, DH], f16, kind="ExternalOutput").ap()
        dWV = nc.dram_tensor("dbg_wv", [P, CC, DH], f16, kind="ExternalOutput").ap()
        dWO = nc.dram_tensor("dbg_wo", [P, 2, D_MODEL], f16, kind="ExternalOutput").ap()
        dXKV = nc.dram_tensor("dbg_xkv", [P, CC, SKV], f16, kind="ExternalOutput").ap()

    with tile.TileContext(nc) as tc:
        with tc.tile_pool(name="res", bufs=1) as res:
            XT_sb = res.tile([P, QB, CC, 512], f16)
            XKV_sb = res.tile([P, CC, SKV], f16)
            WQT_sb = res.tile([P, 2, CC, P], f16)
            WKT_sb = res.tile([P, CC, DH], f16)
            WVT_sb = res.tile([P, CC, DH], f16)
            WOT_sb = res.tile([P, 2, D_MODEL], f16)
            bq_sb = res.tile([P, 2], f32)
            bk_sb = res.tile([P, 2], f32)
            QT_sb = res.tile([P, 2, S], f16)
            KT_sb = res.tile([P, 2, SKV], f16)
            V_sb = res.tile([P, KC, NH_LOC, 65], f16)
            P_sb = res.tile([P, 2, KC, 512], f16)   # parity-double-buffered
            OT_sb = res.tile([P, 2, S], f16)
            vf_sb = res.tile([P, KC], f32)
            ones4 = res.tile([P, NH_LOC, 1], f16)
            nc.vector.memset(ones4[:], 1.0)

            # First wave of DMAs only: what Q-projection (t=0) of block 0
            # needs. Consumers appear to wait on all DMA traffic issued
            # before them, so later inputs are issued just-in-time below.
            nc.sync.dma_start(WQT_sb[:, 0], WQT_d[0])
            nc.sync.dma_start(bq_sb[:], bq_d)
            nc.sync.dma_start(XT_sb[:, 0], XT_d[0])
            nc.sync.dma_start(WQT_sb[:, 1], WQT_d[1])

            with tc.tile_pool(name="osb", bufs=4) as osb, \
                 tc.tile_pool(name="nrm", bufs=4) as nrm, \
                 tc.tile_pool(name="psS", bufs=2, space="PSUM") as psS, \
                 tc.tile_pool(name="psO", bufs=2, space="PSUM") as psO, \
                 tc.tile_pool(name="gen", bufs=2, space="PSUM") as gen:

                # ---------- filler unit emitters ----------
                def emit_qproj(qb, t):
                    psq = gen.tile([P, 512], f32, tag="g")
                    for cc in range(CC):
                        nc.tensor.matmul(
                            psq[:],
                            WQT_sb[:, t, cc, :],
                            XT_sb[:, qb, cc, :],
                            start=(cc == 0), stop=(cc == CC - 1))
                    nc.vector.tensor_scalar_add(
                        QT_sb[:, t, qb * 512:(qb + 1) * 512], psq[:],
                        bq_sb[:, t:t + 1])

                def emit_kproj(k0, sz, t):
                    psk = gen.tile([P, 512], f32, tag="g")
                    for cc in range(CC):
                        nc.tensor.matmul(
                            psk[:, :sz],
                            WKT_sb[:, cc, t * P:(t + 1) * P],
                            XKV_sb[:, cc, k0:k0 + sz],
                            start=(cc == 0), stop=(cc == CC - 1))
                    nc.vector.tensor_scalar_add(
                        KT_sb[:, t, k0:k0 + sz], psk[:, :sz], bk_sb[:, t:t + 1])

                def emit_vproj(kc):
                    psv = gen.tile([P, 512], f32, tag="g")
                    for cc in range(CC):
                        nc.tensor.matmul(
                            psv[:, :DH],
                            XKV_sb[:, cc, kc * P:(kc + 1) * P],
                            WVT_sb[:, cc, :],
                            start=(cc == 0), stop=(cc == CC - 1))
                    nc.vector.tensor_copy(
                        V_sb[:, kc, :, 0:64],
                        psv[:, :DH].rearrange("p (h d) -> p h d", h=NH_LOC))
                    # valid-flag column via DVE (a direct DMA into the
                    # interleaved stride-65 slots clobbers neighboring V
                    # elements: DMA write granule > element size)
                    nc.vector.tensor_scalar_mul(
                        V_sb[:, kc, :, 64:65], ones4[:], vf_sb[:, kc:kc + 1])

                def emit_ph3(qb, qi, nb):
                    # output chunk [128 q, 512 d] for q-chunk qc of block qb
                    qc = qb * 4 + qi
                    ps3 = gen.tile([P, 512], f32, tag="g")
                    for t in range(2):
                        nc.tensor.matmul(
                            ps3[:],
                            OT_sb[:, t, qc * P:(qc + 1) * P],
                            WOT_sb[:, t, nb * 512:(nb + 1) * 512],
                            start=(t == 0), stop=(t == 1))
                    ob = osb.tile([P, 512], f16, tag="ob")
                    if (qi + nb) % 2:
                        nc.scalar.activation(
                            ob[:], ps3[:], mybir.ActivationFunctionType.Copy)
                    else:
                        nc.vector.tensor_copy(ob[:], ps3[:])
                    nc.sync.dma_start(
                        OUT_d[qc * P:(qc + 1) * P, nb * 512:(nb + 1) * 512],
                        ob[:])

                # ---------- attention unit ----------
                def emit_unit(h, qb, fillers):
                    """fillers: list of thunks to emit between scores and AV."""
                    t, po = h // 2, (h % 2) * 64
                    par = (qb * NH_LOC + h) % 2
                    q0 = qb * 512
                    # kc pairs so each Act exp instruction covers 2 kc tiles
                    prs = [list(range(j, min(j + 2, KC)))
                           for j in range(0, KC, 2)]

                    def emit_sc(pair):
                        pss = psS.tile([P, 2, 512], f32, tag="s")
                        for j, kc in enumerate(pair):
                            nc.tensor.matmul(
                                pss[:, j, :],
                                KT_sb[po:po + 64, t, kc * P:(kc + 1) * P],
                                QT_sb[po:po + 64, t, q0:q0 + 512],
                                start=True, stop=True)
                        nc.scalar.activation(
                            P_sb[:, par, pair[0]:pair[0] + len(pair), :],
                            pss[:, 0:len(pair), :],
                            mybir.ActivationFunctionType.Exp, scale=0.125)

                    for pair in prs[:2]:
                        emit_sc(pair)
                    for f in fillers:
                        f()
                    for pair in prs[2:]:
                        emit_sc(pair)

                    pso = psO.tile([65, 512], f32, tag="o")
                    for kc in range(KC):
                        nc.tensor.matmul(
                            pso[:],
                            V_sb[:, kc, h, :],
                            P_sb[:, par, kc, :],
                            start=(kc == 0), stop=(kc == KC - 1))
                    den = nrm.tile([1, 512], f32, tag="den")
                    nc.vector.tensor_copy(den[:], pso[64:65, :])
                    rec = nrm.tile([1, 512], f32, tag="rec")
                    nc.vector.reciprocal_approx_fast(rec[:], den[:])
                    recb = nrm.tile([64, 512], f32, tag="recb")
                    nc.gpsimd.partition_broadcast(recb[:], rec[:], channels=64)
                    nc.vector.tensor_mul(
                        OT_sb[po:po + 64, t, q0:q0 + 512], pso[0:64, :], recb[:])

                # ---------- lead (DMA issues interleaved just-in-time) ----
                nc.sync.dma_start(WKT_sb[:], WKT_d)
                nc.sync.dma_start(bk_sb[:], bk_d)
                nc.sync.dma_start(XKV_sb[:], XKV_d)
                for t in range(2):
                    emit_qproj(0, t)
                nc.sync.dma_start(WVT_sb[:], WVT_d)
                nc.sync.dma_start(vf_sb[:], vf_d)
                for t in range(2):
                    for (k0, sz) in kbs:
                        emit_kproj(k0, sz, t)
                nc.sync.dma_start(WOT_sb[:], WOT_d)
                nc.sync.dma_start(XT_sb[:, 1], XT_d[1])
                for kc in range(KC):
                    emit_vproj(kc)
                for qb in range(2, QB):
                    nc.sync.dma_start(XT_sb[:, qb], XT_d[qb])

                # ---------- main loop ----------
                for qb in range(QB):
                    # build filler thunks for this q-block
                    fillers = []
                    if qb + 1 < QB:
                        for t in range(2):
                            fillers.append(
                                lambda qb=qb, t=t: emit_qproj(qb + 1, t))
                    if qb >= 1:
                        for qi in range(4):
                            for nb in range(2):
                                fillers.append(
                                    lambda qb=qb, qi=qi, nb=nb:
                                        emit_ph3(qb - 1, qi, nb))
                    # spread fillers across the 4 head-units
                    nf = len(fillers)
                    for h in range(NH_LOC):
                        lo = nf * h // NH_LOC
                        hi = nf * (h + 1) // NH_LOC
                        emit_unit(h, qb, fillers[lo:hi])

                # tail: output projection of the last q-block
                for qi in range(4):
                    for nb in range(2):
                        emit_ph3(QB - 1, qi, nb)

                if debug:
                    nc.sync.dma_start(dKT, KT_sb[:])
                    nc.sync.dma_start(dQT, QT_sb[:])
                    nc.sync.dma_start(dV, V_sb[:])
                    nc.sync.dma_start(dOT, OT_sb[:])
                    nc.sync.dma_start(dWK, WKT_sb[:])
                    nc.sync.dma_start(dWV, WVT_sb[:])
                    nc.sync.dma_start(dWO, WOT_sb[:])
                    nc.sync.dma_start(dXKV, XKV_sb[:])

    nc.compile()
    return nc


def kernel(X, mask, W_Q, b_Q, W_K, b_K, W_V, b_V, W_O, b_O):
    global last_results
    from concourse.bass_utils import run_bass_kernel_spmd

    X = np.asarray(X, dtype=np.float32)
    mask2 = np.asarray(mask).reshape(B, S) != 0
    counts = mask2.sum(axis=1)
    assert counts.min() >= 1
    SKV = max(P, int(math.ceil(counts.max() / P)) * P)

    KC = SKV // P
    XT16 = np.ascontiguousarray(X.transpose(0, 2, 1)).astype(np.float16)
    XKV16 = np.zeros((B, D_MODEL, SKV), dtype=np.float16)
    VF32 = np.zeros((B, SKV), dtype=np.float32)
    for b in range(B):
        idx = np.nonzero(mask2[b])[0]
        XKV16[b, :, :len(idx)] = XT16[b][:, idx]
        VF32[b, :len(idx)] = 1.0

    nc = _build(SKV)

    # rearrange to the SBUF layouts (fat contiguous DMA rows)
    def pcd(w):     # [D, n] -> [P, CC, n]
        return np.ascontiguousarray(
            w.reshape(CC, P, w.shape[1]).transpose(1, 0, 2))

    xt_h = [np.ascontiguousarray(
        XT16[b].reshape(CC, P, QB, 512).transpose(2, 1, 0, 3)) for b in range(B)]
    xkv_h = [pcd(XKV16[b]) for b in range(B)]
    vf_h = [np.ascontiguousarray(VF32[b].reshape(KC, P).T) for b in range(B)]

    in_maps = []
    for c in range(N_CORES):
        b, g = divmod(c, GROUPS)
        sl = slice(g * DH, (g + 1) * DH)
        in_maps.append({
            "xt": xt_h[b],
            "xkv": xkv_h[b],
            "wqt": np.ascontiguousarray(
                W_Q[sl, :].T.astype(np.float16)
                .reshape(CC, P, 2, P).transpose(2, 1, 0, 3)),
            "wkt": pcd(W_K[sl, :].T.astype(np.float16)),
            "wvt": pcd(W_V[sl, :].T.astype(np.float16)),
            "wot": np.ascontiguousarray(
                W_O[:, sl].T.astype(np.float16).reshape(2, P, D_MODEL)
                .transpose(1, 0, 2)),
            "bq": np.ascontiguousarray(
                b_Q[sl].astype(np.float32).reshape(2, P).T),
            "bk": np.ascontiguousarray(
                b_K[sl].astype(np.float32).reshape(2, P).T),
            "vf": vf_h[b],
        })

    trace_cores = None
    if os.environ.get("BASS_TRACE"):
        trace_cores = [int(x) for x in
                       os.environ.get("BASS_TRACE_CORES", "0").split(",")]
    res = run_bass_kernel_spmd(nc, in_maps, core_ids=list(range(N_CORES)),
                               trace_cores=trace_cores)
    last_results = res

    const = np.asarray(b_V, np.float64) @ np.asarray(W_O, np.float64).T \
        + np.asarray(b_O, np.float64)
    out = np.zeros((B, S, D_MODEL), dtype=np.float64)
    for c in range(N_CORES):
        b = c // GROUPS
        out[b] += res.results[c]["out"].astype(np.float64)
    out += const[None, None, :]
    return out.astype(np.float32)


# revision 32
# speedup vs baseline: 1.0313x; 1.0237x over previous
"""Multi-head attention (B=2, S=2048, D=1024, H=16) on 8 TRN2 NeuronCores.

Sharding: core c handles batch b = c//4 and head-group g = c%4 (4 heads,
d-slice of 256). All on-chip data is fp16 (PE rate is identical to f32r at
N>=256, but DMA and SBUF cost halve); matmul accumulation stays fp32 in PSUM.

Per core:
  KT = (WkT.T @ Xkv^T + bk)        [256, SKV]  fp16
  V  = Xkv^T.T-chunks @ WvT        [SKV, 256]  fp16 ([k, d] layout + valid col)
  QT = (WqT.T @ X^T + bq)          [256, 2048] fp16
  per (head h, q-block of 512):
    per kc (128 keys): pss = K_h Q_h^T (PSUM), P16 = exp(pss/8) (Act, fp16)
    psO [65, 512] accumulates [V_h | valid].T P16 over kc -> unnormalized O^T
      (row 64 = softmax denominator over valid keys)
    OT = psO[0:64] * recip(den)  (DVE recip + GpSimd partition broadcast)
  OUT[qc, :] = OT.T @ WoT  per 128-row q-chunk, fp16 out, DMA per 512-col half

The emission order software-pipelines the engines: Q-projection of the next
q-block and the output projection of the previous q-block are interleaved
between attention matmuls so the PE never idles while the Act engine works
through the exp stream.

Host side: keys/values are compacted by the attention mask (exact: masked
keys contribute exp->0 in the fp32 reference), padded to a multiple of 128;
the valid-flag column excludes padding from numerator and denominator.
V/O biases fold into a host-side constant: A@(V+bv)Wo^T + bo = A@V@Wo^T +
(bv@Wo^T + bo). Partial outputs over head-groups are summed on the host.
"""

import math
import os
from functools import lru_cache

import numpy as np

D_MODEL = 1024
NUM_HEADS = 16
D_K = 64
B = 2
S = 2048
N_CORES = 8
GROUPS = 4          # head-groups = cores per batch
DH = 256            # d-slice per core (4 heads x 64)
NH_LOC = 4          # heads per core
P = 128
CC = D_MODEL // P   # contraction chunks
QB = S // 512       # q blocks

# results of the last hardware run (BassKernelResults), for test harnesses
last_results = None


@lru_cache(maxsize=2)
def _build(SKV: int):
    import concourse.mybir as mybir
    import concourse.tile as tile
    from concourse import bacc

    f32 = mybir.dt.float32
    f16 = mybir.dt.float16
    KC = SKV // P
    kbs = [(s0, min(512, SKV - s0)) for s0 in range(0, SKV, 512)]

    nc = bacc.Bacc("TRN2", target_bir_lowering=False, debug=False,
                   num_devices=N_CORES)

    # All inputs are pre-arranged on the host to the exact SBUF layout, so
    # every DMA is 128 fat contiguous rows (descriptor generation on the
    # Sync engine is proportional to row count).
    XT_d = nc.dram_tensor("xt", [QB, P, CC, 512], f16, kind="ExternalInput").ap()
    XKV_d = nc.dram_tensor("xkv", [P, CC, SKV], f16, kind="ExternalInput").ap()
    WQT_d = nc.dram_tensor("wqt", [2, P, CC, P], f16, kind="ExternalInput").ap()
    WKT_d = nc.dram_tensor("wkt", [P, CC, DH], f16, kind="ExternalInput").ap()
    WVT_d = nc.dram_tensor("wvt", [P, CC, DH], f16, kind="ExternalInput").ap()
    WOT_d = nc.dram_tensor("wot", [P, 2, D_MODEL], f16, kind="ExternalInput").ap()
    bq_d = nc.dram_tensor("bq", [P, 2], f32, kind="ExternalInput").ap()
    bk_d = nc.dram_tensor("bk", [P, 2], f32, kind="ExternalInput").ap()
    vf_d = nc.dram_tensor("vf", [P, KC], f32, kind="ExternalInput").ap()
    OUT_d = nc.dram_tensor("out", [S, D_MODEL], f16, kind="ExternalOutput").ap()
    debug = bool(os.environ.get("KERNEL_DEBUG"))
    if debug:
        dKT = nc.dram_tensor("dbg_kt", [P, 2, SKV], f16, kind="ExternalOutput").ap()
        dQT = nc.dram_tensor("dbg_qt", [P, 2, S], f16, kind="ExternalOutput").ap()
        dV = nc.dram_tensor("dbg_v", [P, KC, NH_LOC, 65], f16, kind="ExternalOutput").ap()
        dOT = nc.dram_tensor("dbg_ot", [P, 2, S], f16, kind="ExternalOutput").ap()
        dWK = nc.dram_tensor("dbg_wk", [P, CC, DH], f16, kind="ExternalOutput").ap()
        dWV = nc.dram_tensor("dbg_wv", [P, CC, DH], f16, kind="ExternalOutput").ap()
        dWO = nc.dram_tensor("dbg_wo", [P, 2, D_MODEL], f16, kind="ExternalOutput").ap()
        dXKV = nc.dram_tensor("dbg_xkv", [P, CC, SKV], f16, kind="ExternalOutput").ap()

    with tile.TileContext(nc) as tc:
        with tc.tile_pool(name="res", bufs=1) as res:
            XT_sb = res.tile([P, QB, CC, 512], f16)
            XKV_sb = res.tile([P, CC, SKV], f16)
            WQT_sb = res.tile([P, 2, CC, P], f16)
            WKT_sb = res.tile([P, CC, DH], f16)
            WVT_sb = res.tile([P, CC, DH], f16)
            WOT_sb = res.tile([P, 2, D_MODEL], f16)
            bq_sb = res.tile([P, 2], f32)
            bk_sb = res.tile([P, 2], f32)
            QT_sb = res.tile([P, 2, S], f16)
            KT_sb = res.tile([P, 2, SKV], f16)
            V_sb = res.tile([P, KC, NH_LOC, 65], f16)
            P_sb = res.tile([P, 2, KC, 512], f16)   # parity-double-buffered
            OT_sb = res.tile([P, 2, S], f16)
            vf_sb = res.tile([P, KC], f32)
            ones4 = res.tile([P, NH_LOC, 1], f16)
            nc.vector.memset(ones4[:], 1.0)

            # First wave of DMAs only: what Q-projection (t=0) of block 0
            # needs. Consumers appear to wait on all DMA traffic issued
            # before them, so later inputs are issued just-in-time below.
            nc.sync.dma_start(WQT_sb[:, 0], WQT_d[0])
            nc.sync.dma_start(bq_sb[:], bq_d)
            nc.sync.dma_start(XT_sb[:, 0], XT_d[0])
            nc.sync.dma_start(WQT_sb[:, 1], WQT_d[1])

            with tc.tile_pool(name="osb", bufs=4) as osb, \
                 tc.tile_pool(name="nrm", bufs=4) as nrm, \
                 tc.tile_pool(name="psS", bufs=2, space="PSUM") as psS, \
                 tc.tile_pool(name="psO", bufs=2, space="PSUM") as psO, \
                 tc.tile_pool(name="gen", bufs=2, space="PSUM") as gen:

                # ---------- filler unit emitters ----------
                def emit_qproj(qb, t):
                    psq = gen.tile([P, 512], f32, tag="g")
                    for cc in range(CC):
                        nc.tensor.matmul(
                            psq[:],
                            WQT_sb[:, t, cc, :],
                            XT_sb[:, qb, cc, :],
                            start=(cc == 0), stop=(cc == CC - 1))
                    nc.vector.tensor_scalar_add(
                        QT_sb[:, t, qb * 512:(qb + 1) * 512], psq[:],
                        bq_sb[:, t:t + 1])

                def emit_kproj(k0, sz, t):
                    psk = gen.tile([P, 512], f32, tag="g")
                    for cc in range(CC):
                        nc.tensor.matmul(
                            psk[:, :sz],
                            WKT_sb[:, cc, t * P:(t + 1) * P],
                            XKV_sb[:, cc, k0:k0 + sz],
                            start=(cc == 0), stop=(cc == CC - 1))
                    nc.vector.tensor_scalar_add(
                        KT_sb[:, t, k0:k0 + sz], psk[:, :sz], bk_sb[:, t:t + 1])

                def emit_vproj(kc):
                    psv = gen.tile([P, 512], f32, tag="g")
                    for cc in range(CC):
                        nc.tensor.matmul(
                            psv[:, :DH],
                            XKV_sb[:, cc, kc * P:(kc + 1) * P],
                            WVT_sb[:, cc, :],
                            start=(cc == 0), stop=(cc == CC - 1))
                    nc.vector.tensor_copy(
                        V_sb[:, kc, :, 0:64],
                        psv[:, :DH].rearrange("p (h d) -> p h d", h=NH_LOC))
                    # valid-flag column via DVE (a direct DMA into the
                    # interleaved stride-65 slots clobbers neighboring V
                    # elements: DMA write granule > element size)
                    nc.vector.tensor_scalar_mul(
                        V_sb[:, kc, :, 64:65], ones4[:], vf_sb[:, kc:kc + 1])

                def emit_ph3(qb, qi, nb):
                    # output chunk [128 q, 512 d] for q-chunk qc of block qb
                    qc = qb * 4 + qi
                    ps3 = gen.tile([P, 512], f32, tag="g")
                    for t in range(2):
                        nc.tensor.matmul(
                            ps3[:],
                            OT_sb[:, t, qc * P:(qc + 1) * P],
                            WOT_sb[:, t, nb * 512:(nb + 1) * 512],
                            start=(t == 0), stop=(t == 1))
                    ob = osb.tile([P, 512], f16, tag="ob")
                    if (qi + nb) % 2:
                        nc.scalar.activation(
                            ob[:], ps3[:], mybir.ActivationFunctionType.Copy)
                    else:
                        nc.vector.tensor_copy(ob[:], ps3[:])
                    nc.sync.dma_start(
                        OUT_d[qc * P:(qc + 1) * P, nb * 512:(nb + 1) * 512],
                        ob[:])

                # ---------- attention unit ----------
                def emit_unit(h, qb, fillers):
                    """fillers: list of thunks to emit between scores and AV."""
                    t, po = h // 2, (h % 2) * 64
                    par = (qb * NH_LOC + h) % 2
                    q0 = qb * 512
                    # kc pairs so each Act exp instruction covers 2 kc tiles
                    prs = [list(range(j, min(j + 2, KC)))
                           for j in range(0, KC, 2)]

                    def emit_sc(pair):
                        pss = psS.tile([P, 2, 512], f32, tag="s")
                        for j, kc in enumerate(pair):
                            nc.tensor.matmul(
                                pss[:, j, :],
                                KT_sb[po:po + 64, t, kc * P:(kc + 1) * P],
                                QT_sb[po:po + 64, t, q0:q0 + 512],
                                start=True, stop=True)
                        nc.scalar.activation(
                            P_sb[:, par, pair[0]:pair[0] + len(pair), :],
                            pss[:, 0:len(pair), :],
                            mybir.ActivationFunctionType.Exp, scale=0.125)

                    for pair in prs[:2]:
                        emit_sc(pair)
                    for f in fillers:
                        f()
                    for pair in prs[2:]:
                        emit_sc(pair)

                    pso = psO.tile([65, 512], f32, tag="o")
                    for kc in range(KC):
                        nc.tensor.matmul(
                            pso[:],
                            V_sb[:, kc, h, :],
                            P_sb[:, par, kc, :],
                            start=(kc == 0), stop=(kc == KC - 1))
                    den = nrm.tile([1, 512], f32, tag="den")
                    nc.vector.tensor_copy(den[:], pso[64:65, :])
                    rec = nrm.tile([1, 512], f32, tag="rec")
                    nc.vector.reciprocal_approx_fast(rec[:], den[:])
                    recb = nrm.tile([64, 512], f32, tag="recb")
                    nc.gpsimd.partition_broadcast(recb[:], rec[:], channels=64)
                    nc.vector.tensor_mul(
                        OT_sb[po:po + 64, t, q0:q0 + 512], pso[0:64, :], recb[:])

                # ---------- lead (DMA issues interleaved just-in-time) ----
                nc.sync.dma_start(WKT_sb[:], WKT_d)
                nc.sync.dma_start(bk_sb[:], bk_d)
                nc.sync.dma_start(XKV_sb[:], XKV_d)
                for t in range(2):
                    emit_qproj(0, t)
                nc.sync.dma_start(WVT_sb[:], WVT_d)
                nc.sync.dma_start(vf_sb[:], vf_d)
                for t in range(2):
                    for (k0, sz) in kbs:
                        emit_kproj(k0, sz, t)
                nc.sync.dma_start(WOT_sb[:], WOT_d)
                nc.sync.dma_start(XT_sb[:, 1], XT_d[1])
                for kc in range(KC):
                    emit_vproj(kc)
                for qb in range(2, QB):
                    nc.sync.dma_start(XT_sb[:, qb], XT_d[qb])

                # ---------- main loop ----------
                # ph3(qb) units are spread as PE fillers over later blocks;
                # qb3 gets extra (it has no next-block Q-projection filler).
                ph3_sched = {1: [(0, qi, nb) for qi in range(4) for nb in range(2)],
                             2: [(1, qi, nb) for qi in range(4) for nb in range(1)],
                             3: [(1, qi, 1) for qi in range(4)]
                                + [(2, qi, nb) for qi in range(4) for nb in range(2)]}
                for qb in range(QB):
                    # build filler thunks for this q-block
                    fillers = []
                    if qb + 1 < QB:
                        for t in range(2):
                            fillers.append(
                                lambda qb=qb, t=t: emit_qproj(qb + 1, t))
                    for (pqb, qi, nb) in ph3_sched.get(qb, []):
                        fillers.append(
                            lambda pqb=pqb, qi=qi, nb=nb:
                                emit_ph3(pqb, qi, nb))
                    # spread fillers across the 4 head-units
                    nf = len(fillers)
                    for h in range(NH_LOC):
                        lo = nf * h // NH_LOC
                        hi = nf * (h + 1) // NH_LOC
                        emit_unit(h, qb, fillers[lo:hi])

                # tail: output projection of the last q-block
                for qi in range(4):
                    for nb in range(2):
                        emit_ph3(QB - 1, qi, nb)

                if debug:
                    nc.sync.dma_start(dKT, KT_sb[:])
                    nc.sync.dma_start(dQT, QT_sb[:])
                    nc.sync.dma_start(dV, V_sb[:])
                    nc.sync.dma_start(dOT, OT_sb[:])
                    nc.sync.dma_start(dWK, WKT_sb[:])
                    nc.sync.dma_start(dWV, WVT_sb[:])
                    nc.sync.dma_start(dWO, WOT_sb[:])
                    nc.sync.dma_start(dXKV, XKV_sb[:])

    nc.compile()
    return nc


def kernel(X, mask, W_Q, b_Q, W_K, b_K, W_V, b_V, W_O, b_O):
    global last_results
    from concourse.bass_utils import run_bass_kernel_spmd

    X = np.asarray(X, dtype=np.float32)
    mask2 = np.asarray(mask).reshape(B, S) != 0
    counts = mask2.sum(axis=1)
    assert counts.min() >= 1
    SKV = max(P, int(math.ceil(counts.max() / P)) * P)

    KC = SKV // P
    XT16 = np.ascontiguousarray(X.transpose(0, 2, 1)).astype(np.float16)
    XKV16 = np.zeros((B, D_MODEL, SKV), dtype=np.float16)
    VF32 = np.zeros((B, SKV), dtype=np.float32)
    for b in range(B):
        idx = np.nonzero(mask2[b])[0]
        XKV16[b, :, :len(idx)] = XT16[b][:, idx]
        VF32[b, :len(idx)] = 1.0

    nc = _build(SKV)

    # rearrange to the SBUF layouts (fat contiguous DMA rows)
    def pcd(w):     # [D, n] -> [P, CC, n]
        return np.ascontiguousarray(
            w.reshape(CC, P, w.shape[1]).transpose(1, 0, 2))

    xt_h = [np.ascontiguousarray(
        XT16[b].reshape(CC, P, QB, 512).transpose(2, 1, 0, 3)) for b in range(B)]
    xkv_h = [pcd(XKV16[b]) for b in range(B)]
    vf_h = [np.ascontiguousarray(VF32[b].reshape(KC, P).T) for b in range(B)]

    in_maps = []
    for c in range(N_CORES):
        b, g = divmod(c, GROUPS)
        sl = slice(g * DH, (g + 1) * DH)
        in_maps.append({
            "xt": xt_h[b],
            "xkv": xkv_h[b],
            "wqt": np.ascontiguousarray(
                W_Q[sl, :].T.astype(np.float16)
                .reshape(CC, P, 2, P).transpose(2, 1, 0, 3)),
            "wkt": pcd(W_K[sl, :].T.astype(np.float16)),
            "wvt": pcd(W_V[sl, :].T.astype(np.float16)),
            "wot": np.ascontiguousarray(
                W_O[:, sl].T.astype(np.float16).reshape(2, P, D_MODEL)
                .transpose(1, 0, 2)),
            "bq": np.ascontiguousarray(
                b_Q[sl].astype(np.float32).reshape(2, P).T),
            "bk": np.ascontiguousarray(
                b_K[sl].astype(np.float32).reshape(2, P).T),
            "vf": vf_h[b],
        })

    trace_cores = None
    if os.environ.get("BASS_TRACE"):
        trace_cores = [int(x) for x in
                       os.environ.get("BASS_TRACE_CORES", "0").split(",")]
    res = run_bass_kernel_spmd(nc, in_maps, core_ids=list(range(N_CORES)),
                               trace_cores=trace_cores)
    last_results = res

    const = np.asarray(b_V, np.float64) @ np.asarray(W_O, np.float64).T \
        + np.asarray(b_O, np.float64)
    out = np.zeros((B, S, D_MODEL), dtype=np.float64)
    for c in range(N_CORES):
        b = c // GROUPS
        out[b] += res.results[c]["out"].astype(np.float64)
    out += const[None, None, :]
    return out.astype(np.float32)


# revision 42
# speedup vs baseline: 1.0362x; 1.0048x over previous
"""Multi-head attention (B=2, S=2048, D=1024, H=16) on 8 TRN2 NeuronCores.

Sharding: core c handles batch b = c//4 and head-group g = c%4 (4 heads,
d-slice of 256). All on-chip data is fp16 (PE rate is identical to f32r at
N>=256, but DMA and SBUF cost halve); matmul accumulation stays fp32 in PSUM.

Per core:
  KT = (WkT.T @ Xkv^T + bk)        [256, SKV]  fp16
  V  = Xkv^T.T-chunks @ WvT        [SKV, 256]  fp16 ([k, d] layout + valid col)
  QT = (WqT.T @ X^T + bq)          [256, 2048] fp16
  per (head h, q-block of 512):
    per kc (128 keys): pss = K_h Q_h^T (PSUM), P16 = exp(pss/8) (Act, fp16)
    psO [65, 512] accumulates [V_h | valid].T P16 over kc -> unnormalized O^T
      (row 64 = softmax denominator over valid keys)
    OT = psO[0:64] * recip(den)  (DVE recip + GpSimd partition broadcast)
  OUT[qc, :] = OT.T @ WoT  per 128-row q-chunk, fp16 out, DMA per 512-col half

The emission order software-pipelines the engines: Q-projection of the next
q-block and the output projection of the previous q-block are interleaved
between attention matmuls so the PE never idles while the Act engine works
through the exp stream.

Host side: keys/values are compacted by the attention mask (exact: masked
keys contribute exp->0 in the fp32 reference), padded to a multiple of 128;
the valid-flag column excludes padding from numerator and denominator.
V/O biases fold into a host-side constant: A@(V+bv)Wo^T + bo = A@V@Wo^T +
(bv@Wo^T + bo). Partial outputs over head-groups are summed on the host.
"""

import math
import os
from functools import lru_cache

import numpy as np

D_MODEL = 1024
NUM_HEADS = 16
D_K = 64
B = 2
S = 2048
N_CORES = 8
GROUPS = 4          # head-groups = cores per batch
DH = 256            # d-slice per core (4 heads x 64)
NH_LOC = 4          # heads per core
P = 128
CC = D_MODEL // P   # contraction chunks
QB = S // 512       # q blocks

# results of the last hardware run (BassKernelResults), for test harnesses
last_results = None


@lru_cache(maxsize=2)
def _build(SKV: int):
    import concourse.mybir as mybir
    import concourse.tile as tile
    from concourse import bacc

    f32 = mybir.dt.float32
    f16 = mybir.dt.float16
    KC = SKV // P
    kbs = [(s0, min(512, SKV - s0)) for s0 in range(0, SKV, 512)]

    nc = bacc.Bacc("TRN2", target_bir_lowering=False, debug=False,
                   num_devices=N_CORES)

    # All inputs are pre-arranged on the host to the exact SBUF layout, so
    # every DMA is 128 fat contiguous rows (descriptor generation on the
    # Sync engine is proportional to row count).
    XT_d = nc.dram_tensor("xt", [QB, P, CC, 512], f16, kind="ExternalInput").ap()
    XKV_d = nc.dram_tensor("xkv", [P, CC, SKV], f16, kind="ExternalInput").ap()
    WQT_d = nc.dram_tensor("wqt", [2, P, CC, P], f16, kind="ExternalInput").ap()
    WKT_d = nc.dram_tensor("wkt", [P, CC, DH], f16, kind="ExternalInput").ap()
    WVT_d = nc.dram_tensor("wvt", [P, CC, DH], f16, kind="ExternalInput").ap()
    WOT_d = nc.dram_tensor("wot", [P, 2, D_MODEL], f16, kind="ExternalInput").ap()
    # bq | bk | vf packed into one small DMA (descriptor count is what
    # costs on the cold DMA path)
    misc_d = nc.dram_tensor("misc", [P, 4 + KC], f32, kind="ExternalInput").ap()
    OUT_d = nc.dram_tensor("out", [S, D_MODEL], f16, kind="ExternalOutput").ap()
    debug = bool(os.environ.get("KERNEL_DEBUG"))
    if debug:
        dKT = nc.dram_tensor("dbg_kt", [P, 2, SKV], f16, kind="ExternalOutput").ap()
        dQT = nc.dram_tensor("dbg_qt", [P, 2, S], f16, kind="ExternalOutput").ap()
        dV = nc.dram_tensor("dbg_v", [P, KC, NH_LOC, 65], f16, kind="ExternalOutput").ap()
        dOT = nc.dram_tensor("dbg_ot", [P, 2, S], f16, kind="ExternalOutput").ap()
        dWK = nc.dram_tensor("dbg_wk", [P, CC, DH], f16, kind="ExternalOutput").ap()
        dWV = nc.dram_tensor("dbg_wv", [P, CC, DH], f16, kind="ExternalOutput").ap()
        dWO = nc.dram_tensor("dbg_wo", [P, 2, D_MODEL], f16, kind="ExternalOutput").ap()
        dXKV = nc.dram_tensor("dbg_xkv", [P, CC, SKV], f16, kind="ExternalOutput").ap()

    with tile.TileContext(nc) as tc:
        with tc.tile_pool(name="res", bufs=1) as res:
            XT_sb = res.tile([P, QB, CC, 512], f16)
            XKV_sb = res.tile([P, CC, SKV], f16)
            WQT_sb = res.tile([P, 2, CC, P], f16)
            WKT_sb = res.tile([P, CC, DH], f16)
            WVT_sb = res.tile([P, CC, DH], f16)
            WOT_sb = res.tile([P, 2, D_MODEL], f16)
            misc_sb = res.tile([P, 4 + KC], f32)
            QT_sb = res.tile([P, 2, S], f16)
            KT_sb = res.tile([P, 2, SKV], f16)
            V_sb = res.tile([P, KC, NH_LOC, 65], f16)
            P_sb = res.tile([P, 2, KC, 512], f16)   # parity-double-buffered
            OT_sb = res.tile([P, 2, S], f16)
            ones4 = res.tile([P, NH_LOC, 1], f16)
            nc.vector.memset(ones4[:], 1.0)

            # First wave of DMAs only: what Q-projection (t=0) of block 0
            # needs. Consumers appear to wait on all DMA traffic issued
            # before them, so later inputs are issued just-in-time below.
            nc.sync.dma_start(WQT_sb[:, 0], WQT_d[0])
            nc.sync.dma_start(XT_sb[:, 0], XT_d[0])
            nc.sync.dma_start(WQT_sb[:, 1], WQT_d[1])
            nc.sync.dma_start(misc_sb[:], misc_d)

            with tc.tile_pool(name="osb", bufs=4) as osb, \
                 tc.tile_pool(name="nrm", bufs=4) as nrm, \
                 tc.tile_pool(name="psS", bufs=2, space="PSUM") as psS, \
                 tc.tile_pool(name="psO", bufs=2, space="PSUM") as psO, \
                 tc.tile_pool(name="gen", bufs=2, space="PSUM") as gen:

                # ---------- filler unit emitters ----------
                def emit_qproj(qb, t):
                    psq = gen.tile([P, 512], f32, tag="g")
                    for cc in range(CC):
                        nc.tensor.matmul(
                            psq[:],
                            WQT_sb[:, t, cc, :],
                            XT_sb[:, qb, cc, :],
                            start=(cc == 0), stop=(cc == CC - 1))
                    nc.vector.tensor_scalar_add(
                        QT_sb[:, t, qb * 512:(qb + 1) * 512], psq[:],
                        misc_sb[:, t:t + 1])

                def emit_kproj(k0, sz, t):
                    psk = gen.tile([P, 512], f32, tag="g")
                    for cc in range(CC):
                        nc.tensor.matmul(
                            psk[:, :sz],
                            WKT_sb[:, cc, t * P:(t + 1) * P],
                            XKV_sb[:, cc, k0:k0 + sz],
                            start=(cc == 0), stop=(cc == CC - 1))
                    nc.vector.tensor_scalar_add(
                        KT_sb[:, t, k0:k0 + sz], psk[:, :sz],
                        misc_sb[:, 2 + t:3 + t])

                def emit_vproj(kc):
                    psv = gen.tile([P, 512], f32, tag="g")
                    for cc in range(CC):
                        nc.tensor.matmul(
                            psv[:, :DH],
                            XKV_sb[:, cc, kc * P:(kc + 1) * P],
                            WVT_sb[:, cc, :],
                            start=(cc == 0), stop=(cc == CC - 1))
                    nc.vector.tensor_copy(
                        V_sb[:, kc, :, 0:64],
                        psv[:, :DH].rearrange("p (h d) -> p h d", h=NH_LOC))
                    # valid-flag column via DVE (a direct DMA into the
                    # interleaved stride-65 slots clobbers neighboring V
                    # elements: DMA write granule > element size)
                    nc.vector.tensor_scalar_mul(
                        V_sb[:, kc, :, 64:65], ones4[:],
                        misc_sb[:, 4 + kc:5 + kc])

                def emit_ph3(qb, qi, nb):
                    # output chunk [128 q, 512 d] for q-chunk qc of block qb
                    qc = qb * 4 + qi
                    ps3 = gen.tile([P, 512], f32, tag="g")
                    for t in range(2):
                        nc.tensor.matmul(
                            ps3[:],
                            OT_sb[:, t, qc * P:(qc + 1) * P],
                            WOT_sb[:, t, nb * 512:(nb + 1) * 512],
                            start=(t == 0), stop=(t == 1))
                    ob = osb.tile([P, 512], f16, tag="ob")
                    if (qi + nb) % 2:
                        nc.scalar.activation(
                            ob[:], ps3[:], mybir.ActivationFunctionType.Copy)
                    else:
                        nc.vector.tensor_copy(ob[:], ps3[:])
                    nc.sync.dma_start(
                        OUT_d[qc * P:(qc + 1) * P, nb * 512:(nb + 1) * 512],
                        ob[:])

                # ---------- attention unit ----------
                def emit_unit(h, qb, fillers):
                    """fillers: list of thunks to emit between scores and AV."""
                    t, po = h // 2, (h % 2) * 64
                    par = (qb * NH_LOC + h) % 2
                    q0 = qb * 512
                    # kc pairs so each Act exp instruction covers 2 kc tiles
                    prs = [list(range(j, min(j + 2, KC)))
                           for j in range(0, KC, 2)]

                    def emit_sc(pair):
                        pss = psS.tile([P, 2, 512], f32, tag="s")
                        for j, kc in enumerate(pair):
                            nc.tensor.matmul(
                                pss[:, j, :],
                                KT_sb[po:po + 64, t, kc * P:(kc + 1) * P],
                                QT_sb[po:po + 64, t, q0:q0 + 512],
                                start=True, stop=True)
                        nc.scalar.activation(
                            P_sb[:, par, pair[0]:pair[0] + len(pair), :],
                            pss[:, 0:len(pair), :],
                            mybir.ActivationFunctionType.Exp, scale=0.125)

                    for pair in prs[:2]:
                        emit_sc(pair)
                    for f in fillers:
                        f()
                    for pair in prs[2:]:
                        emit_sc(pair)

                    pso = psO.tile([65, 512], f32, tag="o")
                    for kc in range(KC):
                        nc.tensor.matmul(
                            pso[:],
                            V_sb[:, kc, h, :],
                            P_sb[:, par, kc, :],
                            start=(kc == 0), stop=(kc == KC - 1))
                    den = nrm.tile([1, 512], f32, tag="den")
                    nc.vector.tensor_copy(den[:], pso[64:65, :])
                    rec = nrm.tile([1, 512], f32, tag="rec")
                    nc.vector.reciprocal_approx_fast(rec[:], den[:])
                    recb = nrm.tile([64, 512], f32, tag="recb")
                    nc.gpsimd.partition_broadcast(recb[:], rec[:], channels=64)
                    nc.vector.tensor_mul(
                        OT_sb[po:po + 64, t, q0:q0 + 512], pso[0:64, :], recb[:])

                # ---------- lead (DMA issues interleaved just-in-time) ----
                nc.sync.dma_start(WKT_sb[:], WKT_d)
                nc.sync.dma_start(XKV_sb[:], XKV_d)
                for t in range(2):
                    emit_qproj(0, t)
                nc.sync.dma_start(WVT_sb[:], WVT_d)
                for t in range(2):
                    for (k0, sz) in kbs:
                        emit_kproj(k0, sz, t)
                nc.sync.dma_start(WOT_sb[:], WOT_d)
                nc.sync.dma_start(XT_sb[:, 1], XT_d[1])
                for kc in range(KC):
                    emit_vproj(kc)
                for qb in range(2, QB):
                    nc.sync.dma_start(XT_sb[:, qb], XT_d[qb])

                # ---------- main loop ----------
                # ph3(qb) units are spread as PE fillers over later blocks;
                # qb3 gets extra (it has no next-block Q-projection filler).
                ph3_sched = {1: [(0, qi, nb) for qi in range(4) for nb in range(2)],
                             2: [(1, qi, nb) for qi in range(4) for nb in range(1)],
                             3: [(1, qi, 1) for qi in range(4)]
                                + [(2, qi, nb) for qi in range(4) for nb in range(2)]}
                for qb in range(QB):
                    # build filler thunks for this q-block
                    fillers = []
                    if qb + 1 < QB:
                        for t in range(2):
                            fillers.append(
                                lambda qb=qb, t=t: emit_qproj(qb + 1, t))
                    for (pqb, qi, nb) in ph3_sched.get(qb, []):
                        fillers.append(
                            lambda pqb=pqb, qi=qi, nb=nb:
                                emit_ph3(pqb, qi, nb))
                    # spread fillers across the 4 head-units
                    nf = len(fillers)
                    for h in range(NH_LOC):
                        lo = nf * h // NH_LOC
                        hi = nf * (h + 1) // NH_LOC
                        emit_unit(h, qb, fillers[lo:hi])

                # tail: output projection of the last q-block
                for qi in range(4):
                    for nb in range(2):
                        emit_ph3(QB - 1, qi, nb)

                if debug:
                    nc.sync.dma_start(dKT, KT_sb[:])
                    nc.sync.dma_start(dQT, QT_sb[:])
                    nc.sync.dma_start(dV, V_sb[:])
                    nc.sync.dma_start(dOT, OT_sb[:])
                    nc.sync.dma_start(dWK, WKT_sb[:])
                    nc.sync.dma_start(dWV, WVT_sb[:])
                    nc.sync.dma_start(dWO, WOT_sb[:])
                    nc.sync.dma_start(dXKV, XKV_sb[:])

    nc.compile()
    return nc


def kernel(X, mask, W_Q, b_Q, W_K, b_K, W_V, b_V, W_O, b_O):
    global last_results
    from concourse.bass_utils import run_bass_kernel_spmd

    X = np.asarray(X, dtype=np.float32)
    mask2 = np.asarray(mask).reshape(B, S) != 0
    counts = mask2.sum(axis=1)
    assert counts.min() >= 1
    SKV = max(P, int(math.ceil(counts.max() / P)) * P)

    KC = SKV // P
    XT16 = np.ascontiguousarray(X.transpose(0, 2, 1)).astype(np.float16)
    XKV16 = np.zeros((B, D_MODEL, SKV), dtype=np.float16)
    VF32 = np.zeros((B, SKV), dtype=np.float32)
    for b in range(B):
        idx = np.nonzero(mask2[b])[0]
        XKV16[b, :, :len(idx)] = XT16[b][:, idx]
        VF32[b, :len(idx)] = 1.0

    nc = _build(SKV)

    # rearrange to the SBUF layouts (fat contiguous DMA rows)
    def pcd(w):     # [D, n] -> [P, CC, n]
        return np.ascontiguousarray(
            w.reshape(CC, P, w.shape[1]).transpose(1, 0, 2))

    xt_h = [np.ascontiguousarray(
        XT16[b].reshape(CC, P, QB, 512).transpose(2, 1, 0, 3)) for b in range(B)]
    xkv_h = [pcd(XKV16[b]) for b in range(B)]
    vf_h = [np.ascontiguousarray(VF32[b].reshape(KC, P).T) for b in range(B)]

    in_maps = []
    for c in range(N_CORES):
        b, g = divmod(c, GROUPS)
        sl = slice(g * DH, (g + 1) * DH)
        in_maps.append({
            "xt": xt_h[b],
            "xkv": xkv_h[b],
            "wqt": np.ascontiguousarray(
                W_Q[sl, :].T.astype(np.float16)
                .reshape(CC, P, 2, P).transpose(2, 1, 0, 3)),
            "wkt": pcd(W_K[sl, :].T.astype(np.float16)),
            "wvt": pcd(W_V[sl, :].T.astype(np.float16)),
            "wot": np.ascontiguousarray(
                W_O[:, sl].T.astype(np.float16).reshape(2, P, D_MODEL)
                .transpose(1, 0, 2)),
            "misc": np.ascontiguousarray(np.concatenate([
                b_Q[sl].astype(np.float32).reshape(2, P).T,
                b_K[sl].astype(np.float32).reshape(2, P).T,
                vf_h[b]], axis=1)),
        })

    trace_cores = None
    if os.environ.get("BASS_TRACE"):
        trace_cores = [int(x) for x in
                       os.environ.get("BASS_TRACE_CORES", "0").split(",")]
    res = run_bass_kernel_spmd(nc, in_maps, core_ids=list(range(N_CORES)),
                               trace_cores=trace_cores)
    last_results = res

    const = np.asarray(b_V, np.float64) @ np.asarray(W_O, np.float64).T \
        + np.asarray(b_O, np.float64)
    out = np.zeros((B, S, D_MODEL), dtype=np.float64)
    for c in range(N_CORES):
        b = c // GROUPS
        out[b] += res.results[c]["out"].astype(np.float64)
    out += const[None, None, :]
    return out.astype(np.float32)


# revision 46
# speedup vs baseline: 1.0867x; 1.0488x over previous
"""Multi-head attention (B=2, S=2048, D=1024, H=16) on 8 TRN2 NeuronCores.

Sharding: core c handles batch b = c//4 and head-group g = c%4 (4 heads,
d-slice of 256). All on-chip data is fp16 (PE rate is identical to f32r at
N>=256, but DMA and SBUF cost halve); matmul accumulation stays fp32 in PSUM.

Per core:
  KT = (WkT.T @ Xkv^T + bk)        [256, SKV]  fp16
  V  = Xkv^T.T-chunks @ WvT        [SKV, 256]  fp16 ([k, d] layout + valid col)
  QT = (WqT.T @ X^T + bq)          [256, 2048] fp16
  per (head h, q-block of 512):
    per kc (128 keys): pss = K_h Q_h^T (PSUM), P16 = exp(pss/8) (Act, fp16)
    psO [65, 512] accumulates [V_h | valid].T P16 over kc -> unnormalized O^T
      (row 64 = softmax denominator over valid keys)
    OT = psO[0:64] * recip(den)  (DVE recip + GpSimd partition broadcast)
  OUT[qc, :] = OT.T @ WoT  per 128-row q-chunk, fp16 out, DMA per 512-col half

The emission order software-pipelines the engines: Q-projection of the next
q-block and the output projection of the previous q-block are interleaved
between attention matmuls so the PE never idles while the Act engine works
through the exp stream.

Host side: keys/values are compacted by the attention mask (exact: masked
keys contribute exp->0 in the fp32 reference), padded to a multiple of 128;
the valid-flag column excludes padding from numerator and denominator.
V/O biases fold into a host-side constant: A@(V+bv)Wo^T + bo = A@V@Wo^T +
(bv@Wo^T + bo). Partial outputs over head-groups are summed on the host.
"""

import math
import os
from functools import lru_cache

import numpy as np

D_MODEL = 1024
NUM_HEADS = 16
D_K = 64
B = 2
S = 2048
N_CORES = 8
GROUPS = 4          # head-groups = cores per batch
DH = 256            # d-slice per core (4 heads x 64)
NH_LOC = 4          # heads per core
P = 128
CC = D_MODEL // P   # contraction chunks
QB = S // 512       # q blocks

# results of the last hardware run (BassKernelResults), for test harnesses
last_results = None


@lru_cache(maxsize=2)
def _build(SKV: int):
    import concourse.mybir as mybir
    import concourse.tile as tile
    from concourse import bacc

    f32 = mybir.dt.float32
    f16 = mybir.dt.float16
    KC = SKV // P
    kbs = [(s0, min(512, SKV - s0)) for s0 in range(0, SKV, 512)]

    nc = bacc.Bacc("TRN2", target_bir_lowering=False, debug=False,
                   num_devices=N_CORES)

    # All inputs are pre-arranged on the host to the exact SBUF layout, so
    # every DMA is 128 fat contiguous rows (descriptor generation on the
    # Sync engine is proportional to row count).
    XT_d = nc.dram_tensor("xt", [QB, P, CC, 512], f16, kind="ExternalInput").ap()
    XKV_d = nc.dram_tensor("xkv", [P, CC, SKV], f16, kind="ExternalInput").ap()
    WQT_d = nc.dram_tensor("wqt", [2, P, CC, P], f16, kind="ExternalInput").ap()
    WKT_d = nc.dram_tensor("wkt", [P, CC, DH], f16, kind="ExternalInput").ap()
    WVT_d = nc.dram_tensor("wvt", [P, CC, DH], f16, kind="ExternalInput").ap()
    WOT_d = nc.dram_tensor("wot", [P, 2, D_MODEL], f16, kind="ExternalInput").ap()
    # bq | bk | vf packed into one small DMA (descriptor count is what
    # costs on the cold DMA path)
    misc_d = nc.dram_tensor("misc", [P, 4 + KC], f32, kind="ExternalInput").ap()
    OUT_d = nc.dram_tensor("out", [S, D_MODEL], f16, kind="ExternalOutput").ap()
    debug = bool(os.environ.get("KERNEL_DEBUG"))
    if debug:
        dKT = nc.dram_tensor("dbg_kt", [P, 2, SKV], f16, kind="ExternalOutput").ap()
        dQT = nc.dram_tensor("dbg_qt", [P, 2, S], f16, kind="ExternalOutput").ap()
        dV = nc.dram_tensor("dbg_v", [P, KC, NH_LOC, 65], f16, kind="ExternalOutput").ap()
        dOT = nc.dram_tensor("dbg_ot", [P, 2, S], f16, kind="ExternalOutput").ap()
        dWK = nc.dram_tensor("dbg_wk", [P, CC, DH], f16, kind="ExternalOutput").ap()
        dWV = nc.dram_tensor("dbg_wv", [P, CC, DH], f16, kind="ExternalOutput").ap()
        dWO = nc.dram_tensor("dbg_wo", [P, 2, D_MODEL], f16, kind="ExternalOutput").ap()
        dXKV = nc.dram_tensor("dbg_xkv", [P, CC, SKV], f16, kind="ExternalOutput").ap()

    with tile.TileContext(nc) as tc:
        with tc.tile_pool(name="res", bufs=1) as res:
            XT_sb = res.tile([P, QB, CC, 512], f16)
            XKV_sb = res.tile([P, CC, SKV], f16)
            WQT_sb = res.tile([P, 2, CC, P], f16)
            WKT_sb = res.tile([P, CC, DH], f16)
            WVT_sb = res.tile([P, CC, DH], f16)
            WOT_sb = res.tile([P, 2, D_MODEL], f16)
            misc_sb = res.tile([P, 4 + KC], f32)
            QT_sb = res.tile([P, 2, S], f16)
            # Two zero-masked K^T copies: scores for head parity e/o use the
            # full 128-partition contraction (sub-128 weight tiles run ~100ns
            # slower per matmul); the other head's partitions are zero so its
            # Q rows contribute nothing.
            KTe_sb = res.tile([P, 2, SKV], f16)
            KTo_sb = res.tile([P, 2, SKV], f16)
            V_sb = res.tile([P, KC, NH_LOC, 65], f16)
            P_sb = res.tile([P, 2, KC, 512], f16)   # parity-double-buffered
            OT_sb = res.tile([P, 2, S], f16)
            ones4 = res.tile([P, NH_LOC, 1], f16)
            nc.vector.memset(ones4[:], 1.0)
            nc.vector.memset(KTe_sb[64:128, :, :], 0.0)
            nc.vector.memset(KTo_sb[0:64, :, :], 0.0)

            # First wave of DMAs only: what Q-projection (t=0) of block 0
            # needs. Consumers appear to wait on all DMA traffic issued
            # before them, so later inputs are issued just-in-time below.
            nc.sync.dma_start(WQT_sb[:, 0], WQT_d[0])
            nc.sync.dma_start(XT_sb[:, 0], XT_d[0])
            nc.sync.dma_start(WQT_sb[:, 1], WQT_d[1])
            nc.sync.dma_start(misc_sb[:], misc_d)

            with tc.tile_pool(name="osb", bufs=4) as osb, \
                 tc.tile_pool(name="nrm", bufs=4) as nrm, \
                 tc.tile_pool(name="psS", bufs=2, space="PSUM") as psS, \
                 tc.tile_pool(name="psO", bufs=2, space="PSUM") as psO, \
                 tc.tile_pool(name="gen", bufs=2, space="PSUM") as gen:

                # ---------- filler unit emitters ----------
                def emit_qproj(qb, t):
                    psq = gen.tile([P, 512], f32, tag="g")
                    for cc in range(CC):
                        nc.tensor.matmul(
                            psq[:],
                            WQT_sb[:, t, cc, :],
                            XT_sb[:, qb, cc, :],
                            start=(cc == 0), stop=(cc == CC - 1))
                    nc.vector.tensor_scalar_add(
                        QT_sb[:, t, qb * 512:(qb + 1) * 512], psq[:],
                        misc_sb[:, t:t + 1])

                def emit_kproj(k0, sz, t):
                    psk = gen.tile([P, 512], f32, tag="g")
                    for cc in range(CC):
                        nc.tensor.matmul(
                            psk[:, :sz],
                            WKT_sb[:, cc, t * P:(t + 1) * P],
                            XKV_sb[:, cc, k0:k0 + sz],
                            start=(cc == 0), stop=(cc == CC - 1))
                    nc.vector.tensor_scalar_add(
                        KTe_sb[0:64, t, k0:k0 + sz], psk[0:64, :sz],
                        misc_sb[0:64, 2 + t:3 + t])
                    nc.vector.tensor_scalar_add(
                        KTo_sb[64:128, t, k0:k0 + sz], psk[64:128, :sz],
                        misc_sb[64:128, 2 + t:3 + t])

                def emit_vproj(kc):
                    psv = gen.tile([P, 512], f32, tag="g")
                    for cc in range(CC):
                        nc.tensor.matmul(
                            psv[:, :DH],
                            XKV_sb[:, cc, kc * P:(kc + 1) * P],
                            WVT_sb[:, cc, :],
                            start=(cc == 0), stop=(cc == CC - 1))
                    nc.vector.tensor_copy(
                        V_sb[:, kc, :, 0:64],
                        psv[:, :DH].rearrange("p (h d) -> p h d", h=NH_LOC))
                    # valid-flag column via DVE (a direct DMA into the
                    # interleaved stride-65 slots clobbers neighboring V
                    # elements: DMA write granule > element size)
                    nc.vector.tensor_scalar_mul(
                        V_sb[:, kc, :, 64:65], ones4[:],
                        misc_sb[:, 4 + kc:5 + kc])

                def emit_ph3(qb, qi, nb):
                    # output chunk [128 q, 512 d] for q-chunk qc of block qb
                    qc = qb * 4 + qi
                    ps3 = gen.tile([P, 512], f32, tag="g")
                    for t in range(2):
                        nc.tensor.matmul(
                            ps3[:],
                            OT_sb[:, t, qc * P:(qc + 1) * P],
                            WOT_sb[:, t, nb * 512:(nb + 1) * 512],
                            start=(t == 0), stop=(t == 1))
                    ob = osb.tile([P, 512], f16, tag="ob")
                    if (qi + nb) % 2:
                        nc.scalar.activation(
                            ob[:], ps3[:], mybir.ActivationFunctionType.Copy)
                    else:
                        nc.vector.tensor_copy(ob[:], ps3[:])
                    nc.sync.dma_start(
                        OUT_d[qc * P:(qc + 1) * P, nb * 512:(nb + 1) * 512],
                        ob[:])

                # ---------- attention unit ----------
                def emit_unit(h, qb, fillers):
                    """fillers: list of thunks to emit between scores and AV."""
                    t, po = h // 2, (h % 2) * 64
                    par = (qb * NH_LOC + h) % 2
                    q0 = qb * 512
                    # kc pairs so each Act exp instruction covers 2 kc tiles
                    prs = [list(range(j, min(j + 2, KC)))
                           for j in range(0, KC, 2)]

                    KTm = KTe_sb if h % 2 == 0 else KTo_sb

                    def emit_sc(pair):
                        pss = psS.tile([P, 2, 512], f32, tag="s")
                        for j, kc in enumerate(pair):
                            nc.tensor.matmul(
                                pss[:, j, :],
                                KTm[:, t, kc * P:(kc + 1) * P],
                                QT_sb[:, t, q0:q0 + 512],
                                start=True, stop=True)
                        nc.scalar.activation(
                            P_sb[:, par, pair[0]:pair[0] + len(pair), :],
                            pss[:, 0:len(pair), :],
                            mybir.ActivationFunctionType.Exp, scale=0.125)

                    for pair in prs[:2]:
                        emit_sc(pair)
                    for f in fillers:
                        f()
                    for pair in prs[2:]:
                        emit_sc(pair)

                    pso = psO.tile([65, 512], f32, tag="o")
                    for kc in range(KC):
                        nc.tensor.matmul(
                            pso[:],
                            V_sb[:, kc, h, :],
                            P_sb[:, par, kc, :],
                            start=(kc == 0), stop=(kc == KC - 1))
                    den = nrm.tile([1, 512], f32, tag="den")
                    nc.vector.tensor_copy(den[:], pso[64:65, :])
                    rec = nrm.tile([1, 512], f32, tag="rec")
                    nc.vector.reciprocal_approx_fast(rec[:], den[:])
                    recb = nrm.tile([64, 512], f32, tag="recb")
                    nc.gpsimd.partition_broadcast(recb[:], rec[:], channels=64)
                    nc.vector.tensor_mul(
                        OT_sb[po:po + 64, t, q0:q0 + 512], pso[0:64, :], recb[:])

                # ---------- lead (DMA issues interleaved just-in-time) ----
                nc.sync.dma_start(WKT_sb[:], WKT_d)
                nc.sync.dma_start(XKV_sb[:], XKV_d)
                for t in range(2):
                    emit_qproj(0, t)
                nc.sync.dma_start(WVT_sb[:], WVT_d)
                for t in range(2):
                    for (k0, sz) in kbs:
                        emit_kproj(k0, sz, t)
                nc.sync.dma_start(WOT_sb[:], WOT_d)
                nc.sync.dma_start(XT_sb[:, 1], XT_d[1])
                for kc in range(KC):
                    emit_vproj(kc)
                for qb in range(2, QB):
                    nc.sync.dma_start(XT_sb[:, qb], XT_d[qb])

                # ---------- main loop ----------
                # ph3(qb) units are spread as PE fillers over later blocks;
                # qb3 gets extra (it has no next-block Q-projection filler).
                ph3_sched = {1: [(0, qi, nb) for qi in range(4) for nb in range(2)],
                             2: [(1, qi, nb) for qi in range(4) for nb in range(1)],
                             3: [(1, qi, 1) for qi in range(4)]
                                + [(2, qi, nb) for qi in range(4) for nb in range(2)]}
                for qb in range(QB):
                    # build filler thunks for this q-block
                    fillers = []
                    if qb + 1 < QB:
                        for t in range(2):
                            fillers.append(
                                lambda qb=qb, t=t: emit_qproj(qb + 1, t))
                    for (pqb, qi, nb) in ph3_sched.get(qb, []):
                        fillers.append(
                            lambda pqb=pqb, qi=qi, nb=nb:
                                emit_ph3(pqb, qi, nb))
                    # spread fillers across the 4 head-units
                    nf = len(fillers)
                    for h in range(NH_LOC):
                        lo = nf * h // NH_LOC
                        hi = nf * (h + 1) // NH_LOC
                        emit_unit(h, qb, fillers[lo:hi])

                # tail: output projection of the last q-block
                for qi in range(4):
                    for nb in range(2):
                        emit_ph3(QB - 1, qi, nb)

                if debug:
                    nc.sync.dma_start(dKT, KT_sb[:])
                    nc.sync.dma_start(dQT, QT_sb[:])
                    nc.sync.dma_start(dV, V_sb[:])
                    nc.sync.dma_start(dOT, OT_sb[:])
                    nc.sync.dma_start(dWK, WKT_sb[:])
                    nc.sync.dma_start(dWV, WVT_sb[:])
                    nc.sync.dma_start(dWO, WOT_sb[:])
                    nc.sync.dma_start(dXKV, XKV_sb[:])

    nc.compile()
    return nc


def kernel(X, mask, W_Q, b_Q, W_K, b_K, W_V, b_V, W_O, b_O):
    global last_results
    from concourse.bass_utils import run_bass_kernel_spmd

    X = np.asarray(X, dtype=np.float32)
    mask2 = np.asarray(mask).reshape(B, S) != 0
    counts = mask2.sum(axis=1)
    assert counts.min() >= 1
    SKV = max(P, int(math.ceil(counts.max() / P)) * P)

    KC = SKV // P
    XT16 = np.ascontiguousarray(X.transpose(0, 2, 1)).astype(np.float16)
    XKV16 = np.zeros((B, D_MODEL, SKV), dtype=np.float16)
    VF32 = np.zeros((B, SKV), dtype=np.float32)
    for b in range(B):
        idx = np.nonzero(mask2[b])[0]
        XKV16[b, :, :len(idx)] = XT16[b][:, idx]
        VF32[b, :len(idx)] = 1.0

    nc = _build(SKV)

    # rearrange to the SBUF layouts (fat contiguous DMA rows)
    def pcd(w):     # [D, n] -> [P, CC, n]
        return np.ascontiguousarray(
            w.reshape(CC, P, w.shape[1]).transpose(1, 0, 2))

    xt_h = [np.ascontiguousarray(
        XT16[b].reshape(CC, P, QB, 512).transpose(2, 1, 0, 3)) for b in range(B)]
    xkv_h = [pcd(XKV16[b]) for b in range(B)]
    vf_h = [np.ascontiguousarray(VF32[b].reshape(KC, P).T) for b in range(B)]

    in_maps = []
    for c in range(N_CORES):
        b, g = divmod(c, GROUPS)
        sl = slice(g * DH, (g + 1) * DH)
        in_maps.append({
            "xt": xt_h[b],
            "xkv": xkv_h[b],
            "wqt": np.ascontiguousarray(
                W_Q[sl, :].T.astype(np.float16)
                .reshape(CC, P, 2, P).transpose(2, 1, 0, 3)),
            "wkt": pcd(W_K[sl, :].T.astype(np.float16)),
            "wvt": pcd(W_V[sl, :].T.astype(np.float16)),
            "wot": np.ascontiguousarray(
                W_O[:, sl].T.astype(np.float16).reshape(2, P, D_MODEL)
                .transpose(1, 0, 2)),
            "misc": np.ascontiguousarray(np.concatenate([
                b_Q[sl].astype(np.float32).reshape(2, P).T,
                b_K[sl].astype(np.float32).reshape(2, P).T,
                vf_h[b]], axis=1)),
        })

    trace_cores = None
    if os.environ.get("BASS_TRACE"):
        trace_cores = [int(x) for x in
                       os.environ.get("BASS_TRACE_CORES", "0").split(",")]
    res = run_bass_kernel_spmd(nc, in_maps, core_ids=list(range(N_CORES)),
                               trace_cores=trace_cores)
    last_results = res

    const = np.asarray(b_V, np.float64) @ np.asarray(W_O, np.float64).T \
        + np.asarray(b_O, np.float64)
    out = np.zeros((B, S, D_MODEL), dtype=np.float64)
    for c in range(N_CORES):
        b = c // GROUPS
        out[b] += res.results[c]["out"].astype(np.float64)
    out += const[None, None, :]
    return out.astype(np.float32)
